# revision 8
# baseline (speedup 1.0000x reference)
"""GNN message-passing aggregator on 8 Trainium2 NeuronCores.

  h = relu(relu(z @ U1 + c1) @ U2 + c2)
  z = segment_sum(relu(relu(y[src] @ W1 + b1) @ W2 + b2), dst)

Strategy:
  * MLP(y[src]) == MLP(y)[src]: compute the pre-MLP once per node (m), then the
    per-edge work collapses to gather m[src] + segment-sum by dst.
  * Edges are sharded by dst ownership (6250 nodes per core) on the host, so the
    segment-sum is core-local: no collectives at all.
  * Per core the dst nodes form 49 windows of 128; a window's edges are packed
    into 128-slot columns.  Each column is gathered from the m table in HBM via
    dma_gather (int16 indices -> the table is addressed as lo/hi halves) and
    accumulated into the window's PSUM tile with a matmul whose stationary
    operand is a selection matrix P[p, n] = (nodeid[p] == n), built on the DVE
    from a host-streamed compact nodeid vector.  PSUM accumulation gives an
    exact fp32 segment sum.
  * Phase 1 (m = pre-MLP(y)) and phase 3 (node-update MLP) run as dense fp32
    matmuls; activations live transposed ([D, nodes]) with a ones-row appended
    so biases fold into the matmuls.
"""

import os

import numpy as np

# ---------------------------------------------------------------- constants
N_NODES = 50000
D = 64
NC = 8                      # cores
W = 128                     # window size == psum partitions
SPLIT = 32768               # int16-addressable half of the m table
OP_COLS = 8                 # gather columns per dma_gather op (8*128=1024 idx)
CHUNK = 512                 # dense-MLP T-major matmul chunk

_COMPILED = {}


def _dims():
    npc = N_NODES // NC
    nw = (npc + W - 1) // W
    npad = nw * W
    mt_rows = ((N_NODES + 127) // 128) * 128
    return npc, nw, npad, mt_rows


# ------------------------------------------------------------ host schedule
def _host_schedule(src, dst):
    """Shard edges by dst ownership, balance nodes into windows, pack columns.

    Returns (sched, per_core): sched is the shared compile-time schedule
    (identical across cores); per_core holds the input arrays per core.
    """
    NPC, NW, NPAD, _ = _dims()

    percore_groups = []
    percore_perm = []
    clo = np.zeros((NC, NW), np.int64)
    chi = np.zeros((NC, NW), np.int64)

    for c in range(NC):
        lo_n, hi_n = c * NPC, (c + 1) * NPC
        sel = (dst >= lo_n) & (dst < hi_n)
        s = src[sel].astype(np.int64)
        d = (dst[sel] - lo_n).astype(np.int64)
        deg = np.bincount(d, minlength=NPC)

        # balance nodes into NW windows by degree (greedy, descending)
        order = np.argsort(-deg, kind="stable")
        wload = np.zeros(NW, np.int64)
        wcount = np.zeros(NW, np.int64)
        assign = np.zeros(NPC, np.int64)
        label = np.zeros(NPC, np.int64)
        for n in order:
            wavail = np.flatnonzero(wcount < W)
            wsel = wavail[np.argmin(wload[wavail])]
            assign[n] = wsel
            label[n] = wcount[wsel]
            wcount[wsel] += 1
            wload[wsel] += deg[n]

        ew = assign[d]
        is_lo = s < SPLIT
        groups = {}
        for wdw in range(NW):
            m_w = ew == wdw
            for t in range(2):
                m_t = m_w & (is_lo if t == 0 else ~is_lo)
                es = s[m_t]
                el = label[d[m_t]]
                o = np.argsort(es, kind="stable")   # src-sorted for HBM locality
                groups[(wdw, t)] = (es[o], el[o])
                cnt = (len(es) + W - 1) // W
                if t == 0:
                    clo[c, wdw] = cnt
                else:
                    chi[c, wdw] = cnt
        percore_groups.append(groups)
        perm = np.full(NPAD, -1, np.int64)
        perm[assign * W + label] = np.arange(NPC) + lo_n
        percore_perm.append(perm)

    CLo = np.maximum(clo.max(0), 1)
    CHi = np.maximum(chi.max(0), 1)
    lo_off = np.concatenate([[0], np.cumsum(CLo)])
    hi_off = np.concatenate([[0], np.cumsum(CHi)])
    n_lo, n_hi = int(lo_off[-1]), int(hi_off[-1])

    per_core = []
    for c in range(NC):
        groups = percore_groups[c]
        idx = [np.zeros(n_lo * W, np.int16), np.zeros(n_hi * W, np.int16)]
        nid = [np.full(n_lo * W, -1.0, np.float32),
               np.full(n_hi * W, -1.0, np.float32)]
        for wdw in range(NW):
            for t, (cnt, off) in enumerate(((CLo, lo_off), (CHi, hi_off))):
                es, el = groups[(wdw, t)]
                base = int(off[wdw]) * W
                nslot = int(cnt[wdw]) * W
                k = len(es)
                idx[t][base:base + k] = (es if t == 0 else es - SPLIT).astype(np.int16)
                nid[t][base:base + k] = el.astype(np.float32)
        # wrap idx into per-op [16, ni/16] layout replicated to 128 partitions
        wrapped = []
        for t, ncols in ((0, n_lo), (1, n_hi)):
            flat = idx[t]
            pos, blocks = 0, []
            while pos < ncols:
                k = min(OP_COLS, ncols - pos)
                ni = k * W
                op = flat[pos * W:(pos + k) * W]
                blk = op.reshape(ni // 16, 16).T            # [16, ni/16]
                blocks.append(np.tile(blk, (8, 1)))          # [128, ni/16]
                pos += k
            wrapped.append(np.concatenate(blocks, axis=1) if blocks
                           else np.zeros((128, 8), np.int16))
        nodeid_2d = np.concatenate([nid[0], nid[1]]).reshape(n_lo + n_hi, W).T
        per_core.append({
            "idx_lo": np.ascontiguousarray(wrapped[0]),
            "idx_hi": np.ascontiguousarray(wrapped[1]),
            "nodeid": np.ascontiguousarray(nodeid_2d.astype(np.float32)),
            "perm": percore_perm[c],
        })

    sched = {"CLo": CLo.astype(int).tolist(), "CHi": CHi.astype(int).tolist(),
             "n_lo": n_lo, "n_hi": n_hi}
    return sched, per_core


# ------------------------------------------------------------- bass program
def _build_program(sched):
    import concourse.bacc as bacc
    import concourse.mybir as mybir
    import concourse.tile as tile
    from concourse.masks import make_identity

    f32 = mybir.dt.float32
    i16 = mybir.dt.int16
    Relu = mybir.ActivationFunctionType.Relu

    NPC, NW, NPAD, MT_ROWS = _dims()
    CLo, CHi = sched["CLo"], sched["CHi"]
    n_lo, n_hi = sched["n_lo"], sched["n_hi"]
    n_cols = n_lo + n_hi

    nc = bacc.Bacc()
    yT_in = nc.dram_tensor("yT", [D + 1, MT_ROWS], f32, kind="ExternalInput")
    wb1_in = nc.dram_tensor("wb1", [D + 1, D], f32, kind="ExternalInput")
    wb2_in = nc.dram_tensor("wb2", [D + 1, D], f32, kind="ExternalInput")
    ub1_in = nc.dram_tensor("ub1", [D + 1, D], f32, kind="ExternalInput")
    ub2_in = nc.dram_tensor("ub2", [D + 1, D], f32, kind="ExternalInput")
    idxlo_in = nc.dram_tensor("idx_lo", [128, n_lo * 8], i16, kind="ExternalInput")
    idxhi_in = nc.dram_tensor("idx_hi", [128, n_hi * 8], i16, kind="ExternalInput")
    nodeid_in = nc.dram_tensor("nodeid", [128, n_cols], f32, kind="ExternalInput")
    iota_in = nc.dram_tensor("iota128", [128, 128], f32, kind="ExternalInput")
    m_dram = nc.dram_tensor("m_scratch", [MT_ROWS, D], f32, kind="Internal")
    h_out = nc.dram_tensor("h_out", [NPAD, D], f32, kind="ExternalOutput")
    debug = bool(int(os.environ.get("KERNEL_DEBUG_Z", "0")))
    if debug:
        z_out = nc.dram_tensor("z_out", [NPAD, D], f32, kind="ExternalOutput")
        m_out = nc.dram_tensor("m_out", [MT_ROWS, D], f32, kind="ExternalOutput")

    with tile.TileContext(nc) as tc:
        with tc.tile_pool(name="const", bufs=1) as cpool:
            wb1 = cpool.tile([D + 1, D], f32, tag="wb1")
            wb2 = cpool.tile([D + 1, D], f32, tag="wb2")
            ub1 = cpool.tile([D + 1, D], f32, tag="ub1")
            ub2 = cpool.tile([D + 1, D], f32, tag="ub2")
            iota = cpool.tile([128, 128], f32, tag="iota")
            ident = cpool.tile([128, 128], f32, tag="ident")
            nc.sync.dma_start(out=wb1[:], in_=wb1_in[:])
            nc.sync.dma_start(out=wb2[:], in_=wb2_in[:])
            nc.sync.dma_start(out=ub1[:], in_=ub1_in[:])
            nc.sync.dma_start(out=ub2[:], in_=ub2_in[:])
            nc.sync.dma_start(out=iota[:], in_=iota_in[:])
            make_identity(nc, ident[:])

            # ------------ phase 1: m = relu(relu(y@W1+b1)@W2+b2) -> m_dram ---
            with tc.tile_pool(name="p1y", bufs=3) as p1y, \
                 tc.tile_pool(name="p1h", bufs=2) as p1h, \
                 tc.tile_pool(name="p1m", bufs=2) as p1m, \
                 tc.tile_pool(name="p1ps", bufs=2, space="PSUM") as p1ps, \
                 tc.tile_pool(name="p1ps2", bufs=2, space="PSUM") as p1ps2:
                for ch in range((MT_ROWS + CHUNK - 1) // CHUNK):
                    c0 = ch * CHUNK
                    cw = min(CHUNK, MT_ROWS - c0)
                    ytile = p1y.tile([D + 1, CHUNK], f32, tag="ytile")
                    nc.sync.dma_start(out=ytile[:, :cw], in_=yT_in[:, c0:c0 + cw])
                    ps = p1ps.tile([D, CHUNK], f32, tag="ps1")
                    nc.tensor.matmul(out=ps[:, :cw], lhsT=wb1[:], rhs=ytile[:, :cw],
                                     start=True, stop=True)
                    h1c = p1h.tile([D + 1, CHUNK], f32, tag="h1c")
                    nc.scalar.activation(out=h1c[:D, :cw], in_=ps[:, :cw], func=Relu)
                    nc.gpsimd.memset(h1c[D:D + 1, :cw], 1.0)
                    mch = p1m.tile([128, (CHUNK // 128) * D], f32, tag="mch")
                    for i in range(cw // 128):
                        ps2 = p1ps2.tile([128, D], f32, tag="ps2")
                        nc.tensor.matmul(out=ps2[:],
                                         lhsT=h1c[:, i * 128:(i + 1) * 128],
                                         rhs=wb2[:], start=True, stop=True)
                        nc.vector.tensor_scalar_max(
                            out=mch[:, i * D:(i + 1) * D], in0=ps2[:], scalar1=0.0)
                    nc.sync.dma_start(
                        out=m_dram[c0:c0 + cw, :].rearrange(
                            "(t p) d -> p t d", p=128),
                        in_=mch[:, :(cw // 128) * D].rearrange(
                            "p (t d) -> p t d", d=D))

            # Tile does not track DRAM RAW deps: order phase-1 m_dram writes
            # before the phase-2 gathers explicitly.
            tc.strict_bb_all_engine_barrier()

            # ------------ phase 2: gather + segment-sum ----------------------
            with tc.tile_pool(name="idxp", bufs=1) as idxp, \
                 tc.tile_pool(name="zpool", bufs=1) as zpool:
                idx_lo_t = idxp.tile([128, n_lo * 8], i16, tag="ilo")
                idx_hi_t = idxp.tile([128, n_hi * 8], i16, tag="ihi")
                idx_t = [idx_lo_t, idx_hi_t]
                nc.sync.dma_start(out=idx_t[0][:], in_=idxlo_in[:])
                nc.sync.dma_start(out=idx_t[1][:], in_=idxhi_in[:])
                nodeid_t = idxp.tile([128, n_cols], f32, tag="nid")
                nc.sync.dma_start(out=nodeid_t[:], in_=nodeid_in[:])
                z_sb = zpool.tile([128, NW * D], f32, tag="z")

                with tc.tile_pool(name="gpool", bufs=6) as gpool, \
                     tc.tile_pool(name="ppool", bufs=4) as ppool, \
                     tc.tile_pool(name="wps", bufs=8, space="PSUM") as wps:
                    tables = [m_dram[0:SPLIT, :], m_dram[SPLIT:MT_ROWS, :]]
                    ncols_t = [n_lo, n_hi]
                    g_tiles = [{}, {}]

                    def ensure_op(t, col):
                        o = col // OP_COLS
                        if o in g_tiles[t]:
                            return g_tiles[t][o]
                        k = min(OP_COLS, ncols_t[t] - o * OP_COLS)
                        g = gpool.tile([128, k, D], f32, tag=f"g{t}")
                        ni = k * W
                        nc.gpsimd.dma_gather(
                            out_ap=g[:], in_ap=tables[t],
                            idxs_ap=idx_t[t][:, o * OP_COLS * 8:o * OP_COLS * 8 + k * 8],
                            num_idxs=ni, num_idxs_reg=ni, elem_size=D)
                        g_tiles[t][o] = g
                        return g

                    lo_base, hi_base = 0, 0
                    for wdw in range(NW):
                        zw = wps.tile([128, D], f32, tag="zw")
                        total = CLo[wdw] + CHi[wdw]
                        ci = 0
                        for t, cnt, base in ((0, CLo[wdw], lo_base),
                                             (1, CHi[wdw], hi_base)):
                            for j in range(cnt):
                                col = base + j
                                g = ensure_op(t, col)
                                sub = col - (col // OP_COLS) * OP_COLS
                                gcol = col if t == 0 else n_lo + col
                                P = ppool.tile([128, 128], f32, tag="P")
                                nc.vector.tensor_tensor(
                                    out=P[:],
                                    in0=nodeid_t[:, gcol:gcol + 1].to_broadcast(
                                        [128, 128]),
                                    in1=iota[:], op=mybir.AluOpType.is_equal)
                                nc.tensor.matmul(out=zw[:], lhsT=P[:],
                                                 rhs=g[:, sub, :],
                                                 start=(ci == 0),
                                                 stop=(ci == total - 1))
                                ci += 1
                        lo_base += CLo[wdw]
                        hi_base += CHi[wdw]
                        nc.scalar.copy(out=z_sb[:, wdw * D:(wdw + 1) * D], in_=zw[:])

                if debug:
                    nc.sync.dma_start(
                        out=z_out[:].rearrange("(t p) d -> p t d", p=128),
                        in_=z_sb[:].rearrange("p (t d) -> p t d", d=D))
                    nc.sync.dma_start(out=m_out[:], in_=m_dram[:])

                # ------------ phase 3: h = relu(relu(z@U1+c1)@U2+c2) ---------
                with tc.tile_pool(name="p3big", bufs=1) as p3big, \
                     tc.tile_pool(name="p3ps", bufs=2, space="PSUM") as p3ps, \
                     tc.tile_pool(name="p3psb", bufs=2, space="PSUM") as p3psb, \
                     tc.tile_pool(name="p3ps2", bufs=2, space="PSUM") as p3ps2:
                    zT = p3big.tile([D + 1, NPAD], f32, tag="zT")
                    nc.gpsimd.memset(zT[D:D + 1, :], 1.0)
                    for wdw in range(NW):
                        pst = p3ps.tile([D, 128], f32, tag="pst")
                        nc.tensor.transpose(out=pst[:],
                                            in_=z_sb[:, wdw * D:(wdw + 1) * D],
                                            identity=ident[:])
                        nc.vector.tensor_copy(out=zT[:D, wdw * 128:(wdw + 1) * 128],
                                              in_=pst[:])
                    g1T = p3big.tile([D + 1, NPAD], f32, tag="g1T")
                    nc.gpsimd.memset(g1T[D:D + 1, :], 1.0)
                    nchunk3 = (NPAD + CHUNK - 1) // CHUNK
                    for ch in range(nchunk3):
                        c0 = ch * CHUNK
                        cw = min(CHUNK, NPAD - c0)
                        ps = p3psb.tile([D, CHUNK], f32, tag="ps3")
                        nc.tensor.matmul(out=ps[:, :cw], lhsT=ub1[:],
                                         rhs=zT[:, c0:c0 + cw],
                                         start=True, stop=True)
                        nc.scalar.activation(out=g1T[:D, c0:c0 + cw],
                                             in_=ps[:, :cw], func=Relu)
                    h_sb = p3big.tile([128, NW * D], f32, tag="h_sb")
                    for wdw in range(NW):
                        ps2 = p3ps2.tile([128, D], f32, tag="ps4")
                        nc.tensor.matmul(out=ps2[:],
                                         lhsT=g1T[:, wdw * 128:(wdw + 1) * 128],
                                         rhs=ub2[:], start=True, stop=True)
                        nc.vector.tensor_scalar_max(
                            out=h_sb[:, wdw * D:(wdw + 1) * D], in0=ps2[:],
                            scalar1=0.0)
                    nc.sync.dma_start(
                        out=h_out[:].rearrange("(t p) d -> p t d", p=128),
                        in_=h_sb[:].rearrange("p (t d) -> p t d", d=D))

    nc.compile()
    return nc


# ------------------------------------------------------------------- kernel
def kernel(**inputs):
    from concourse.bass_utils import run_bass_kernel_spmd

    NPC, NW, NPAD, MT_ROWS = _dims()
    y = np.asarray(inputs["y"], np.float32)
    src = np.asarray(inputs["src"])
    dst = np.asarray(inputs["dst"])
    Ws = {k: np.asarray(inputs[k], np.float32)
          for k in ("W1", "b1", "W2", "b2", "U1", "c1", "U2", "c2")}

    sched, per_core = _host_schedule(src, dst)
    key = (tuple(sched["CLo"]), tuple(sched["CHi"]))
    if key not in _COMPILED:
        _COMPILED[key] = _build_program(sched)
    nc = _COMPILED[key]

    yT = np.zeros((D + 1, MT_ROWS), np.float32)
    yT[:D, :N_NODES] = y.T
    yT[D, :] = 1.0
    wb1 = np.concatenate([Ws["W1"], Ws["b1"][None, :]], axis=0)
    wb2 = np.concatenate([Ws["W2"], Ws["b2"][None, :]], axis=0)
    ub1 = np.concatenate([Ws["U1"], Ws["c1"][None, :]], axis=0)
    ub2 = np.concatenate([Ws["U2"], Ws["c2"][None, :]], axis=0)
    iota = np.tile(np.arange(128, dtype=np.float32), (128, 1))

    in_maps = []
    for c in range(NC):
        pc = per_core[c]
        in_maps.append({
            "yT": yT, "wb1": wb1, "wb2": wb2, "ub1": ub1, "ub2": ub2,
            "idx_lo": pc["idx_lo"], "idx_hi": pc["idx_hi"],
            "nodeid": pc["nodeid"], "iota128": iota,
        })

    res = run_bass_kernel_spmd(nc, in_maps, core_ids=list(range(NC)),
                               trace=bool(int(os.environ.get("KERNEL_TRACE", "0"))))
    kernel.last_results = res
    kernel.last_exec_time_ns = res.exec_time_ns

    h_full = np.zeros((N_NODES, D), np.float32)
    for c in range(NC):
        out = res.results[c]["h_out"]
        perm = per_core[c]["perm"]
        valid = perm >= 0
        h_full[perm[valid]] = out[valid]
    return h_full
